# revision 9
# baseline (speedup 1.0000x reference)
"""Trainium2 Bass kernel for nn_NewDAGExecutor (plan-predictor matmul + 8-step DAG).

Strategy (8 NeuronCores, data-parallel over the 16384 tokens, 2048 tokens/core):
  - Host: split fp32 into an exact fp16 hi/lo pair (lo scaled by 2^11) for both
    the token matrix and the concatenated weight matrix W = [W_init; W_op;
    W_gate] (168 outputs). Layouts are partition-contiguous so DMA descriptors
    are 8KB+ per partition.
  - Device: plan = hidden @ W.T + b as 3 fp16 matmul passes per k-chunk
    (hi*[Wh|Wl] -> p12[0:336], lo*Wh accumulated into the hi*Wl half), which is
    fp32-accurate. Tokens ride the PSUM partition axis. PSUM eviction/combine
    runs on ACT (two copies) + GpSimd (add), keeping DVE free.
  - The 8 sequential DAG steps run wide across 1024 tokens/chunk on DVE/ACT,
    with chunk A's DAG overlapping chunk B's matmuls. Per-step running-dot
    tables (TL) replace wide multiply+reduce; the sign-product tree runs on
    GpSimd; tanh(x*1e4) = (1-e)/(1+e) with e = exp(-2e4*clip(x)) so every ACT
    call sits in the natural_log_exp table set (single table load).
"""

import numpy as np

import concourse.bacc as bacc
import concourse.bass as bass
import concourse.tile as tile
import concourse.mybir as mybir
from concourse.bass_utils import run_bass_kernel_spmd

# Pin Ln/Exp to natural_log_exp_and_others so the kernel needs exactly one
# activation-table load (the greedy first-match in insert_act_table_loads
# would otherwise pick tables per-call).
_ORIG_GAT = bacc.get_activation_tables


def _pinned_activation_tables(arch):
    tables = _ORIG_GAT(arch)
    LN = mybir.ActivationFunctionType.Ln
    EXP = mybir.ActivationFunctionType.Exp
    for name, funcs in tables.items():
        if name != "natural_log_exp_and_others":
            funcs.discard(LN)
            funcs.discard(EXP)
    return tables


bacc.get_activation_tables = _pinned_activation_tables

F32 = mybir.dt.float32
F16 = mybir.dt.float16
U32 = mybir.dt.uint32
ALU = mybir.AluOpType
ACTF = mybir.ActivationFunctionType
AXX = mybir.AxisListType.X

NCORES = 8
B, T, H = 4, 4096, 2048
NTOK = B * T                    # 16384
TPC = NTOK // NCORES            # 2048 tokens per core
NTILE = TPC // 128              # 16 token tiles per core
KCH = H // 128                  # 16 contraction chunks
NN = 16                         # DAG nodes
INTER = 8                       # steps
INIT_SLOTS = 8
NF = 168                        # 32 init + 128 op + 8 gate
LOG_CLAMP = 23.026
SCL = 2048.0                    # 2^11 lo-part scale
ISCL = 1.0 / SCL
NCHUNKS = 2
CW = NTILE // NCHUNKS           # token columns per chunk (8)
CLIP = 1.5e-3                   # |x| <= CLIP -> |exp arg| <= 30

_CACHE = {}


def _ap(t, offset, axes):
    return bass.AP(tensor=t.tensor, offset=t.offset + offset, ap=[t.ap[0]] + axes)


def _build():
    nc = bacc.Bacc("TRN2", target_bir_lowering=False, debug=False)

    hf_d = nc.dram_tensor("hf", [NTILE, 128, KCH * 256], F16, kind="ExternalInput")
    wt_d = nc.dram_tensor("wt", [128, KCH * 336], F16, kind="ExternalInput")
    bias_d = nc.dram_tensor("bias", [1, 2 * NF], F16, kind="ExternalInput")
    out_d = nc.dram_tensor("out", [128, NTILE], F32, kind="ExternalOutput")

    with tile.TileContext(nc) as tc:
        with tc.tile_pool(name="consts", bufs=1) as consts, \
             tc.tile_pool(name="hfp", bufs=3) as hfp, \
             tc.tile_pool(name="evp", bufs=3) as evp, \
             tc.tile_pool(name="ns", bufs=2) as ns, \
             tc.tile_pool(name="gp", bufs=2) as gp, \
             tc.tile_pool(name="pp", bufs=3, space="PSUM") as pp:

            wt_sb = consts.tile([128, KCH, 336], F16)
            nc.sync.dma_start(out=wt_sb, in_=wt_d.rearrange("p (k f) -> p k f", k=KCH))
            bias_sb = consts.tile([1, 2 * NF], F16)
            nc.sync.dma_start(out=bias_sb, in_=bias_d[:, :])
            ones = consts.tile([1, 128], F16)
            nc.vector.memset(ones, 1.0)
            eps = consts.tile([128, 1], F32)
            nc.vector.memset(eps, 1e-12)
            cmask = consts.tile([128, 1], F32)
            nc.vector.memset(cmask, -1.0)
            cone = consts.tile([128, 1], F32)
            nc.vector.memset(cone, 1.0)

            plan = [consts.tile([128, CW, NF], F32, name=f"plan{c}") for c in range(2)]
            st = [None, None]

            for i in range(NTILE):
                c, col = i // CW, i % CW
                hf_sb = hfp.tile([128, KCH, 256], F16, tag="hf")
                nc.sync.dma_start(out=hf_sb, in_=hf_d[i].rearrange("p (k ct) -> p k ct", k=KCH))
                p12 = pp.tile([128, 2 * NF], F32, tag="p12")
                nc.tensor.matmul(p12, ones[:, :], bias_sb[:, :], start=True, stop=False)
                for k in range(KCH):
                    hi = hf_sb[:, k, 0:128]
                    lo = hf_sb[:, k, 128:256]
                    wh = wt_sb[:, k, 0:NF]
                    whl = wt_sb[:, k, :]
                    nc.tensor.matmul(p12, hi, whl, start=False, stop=False)
                    nc.tensor.matmul(p12[:, NF:2 * NF], lo, wh, start=False,
                                     stop=(k == KCH - 1), skip_group_check=True)

                if i >= 8:
                    _dag_step(nc, ns, gp, plan[0], st[0], 0, i - 8)

                # PSUM eviction: 2x ACT copy + GpSimd add (keeps DVE free)
                tmph = evp.tile([128, NF], F32, tag="evh")
                nc.scalar.activation(tmph, p12[:, 0:NF], ACTF.Copy)
                tmpv = evp.tile([128, NF], F32, tag="evl")
                nc.scalar.activation(tmpv, p12[:, NF:2 * NF], ACTF.Copy, scale=ISCL)
                nc.gpsimd.tensor_tensor(out=plan[c][:, col, :], in0=tmph, in1=tmpv, op=ALU.add)

                if i == 7:
                    st[0] = _dag_init(nc, consts, ns, plan[0], 0, eps, cmask, cone)

            nc.sync.dma_start(out=out_d[:, 0:CW], in_=st[0]["OUT"])

            st[1] = _dag_init(nc, consts, ns, plan[1], 1, eps, cmask, cone)
            for s in range(INTER):
                _dag_step(nc, ns, gp, plan[1], st[1], 1, s)
            nc.sync.dma_start(out=out_d[:, CW:NTILE], in_=st[1]["OUT"])

    nc.compile()
    return nc


def _dag_init(nc, consts, ns, PLAN, c, eps, cmask, cone):
    """Init-core for one chunk: VSIGN/G via exp-math, LMD init, pv/PVT, TL."""
    st = {}
    cw = CW
    G = st["G"] = consts.tile([128, cw, INTER], F32, name=f"G{c}")
    VSIGN = st["VSIGN"] = consts.tile([128, cw, NN], F32, name=f"VSIGN{c}")
    LMD = st["LMD"] = consts.tile([128, 2, cw, NN], F32, name=f"LMD{c}")
    TL = st["TL"] = consts.tile([128, 2, INTER, cw], F32, name=f"TL{c}")
    PVT = st["PVT"] = consts.tile([128, cw, INTER], F32, name=f"PVT{c}")
    AONE = st["AONE"] = consts.tile([128, cw, NN], F32, name=f"AONE{c}")
    st["OUT"] = consts.tile([128, cw], F32, name=f"OUT{c}")
    st["CMASK"] = cmask
    st["CONE"] = cone

    def T(nm, shape, dt=F32):
        return ns.tile(shape, dt, tag=f"{nm}{c}", name=f"{nm}{c}")

    nc.gpsimd.memset(AONE, 1.0)

    # VSIGN = tanh(plan[16:32]) = (1-e)/(1+e), e = exp(-2*clip(x, +-15))
    vc = T("vc", [128, cw, NN])
    nc.vector.tensor_scalar(out=vc, in0=PLAN[:, :, 16:32], scalar1=-15.0, scalar2=15.0,
                            op0=ALU.max, op1=ALU.min)
    ev = T("ev", [128, cw, NN])
    nc.scalar.activation(ev, vc, ACTF.Exp, bias=0.0, scale=-2.0)
    nv = T("nv", [128, cw, NN])
    nc.vector.tensor_scalar(out=nv, in0=ev, scalar1=-1.0, scalar2=1.0, op0=ALU.mult, op1=ALU.add)
    dv = T("dv", [128, cw, NN])
    nc.vector.tensor_scalar(out=dv, in0=ev, scalar1=1.0, scalar2=None, op0=ALU.add)
    rv = T("rv", [128, cw, NN])
    nc.vector.reciprocal(out=rv, in_=dv)
    nc.vector.tensor_tensor(out=VSIGN, in0=nv, in1=rv, op=ALU.mult)

    # G = sigmoid(plan[160:168]) = 1/(1+exp(-x))
    gc = T("gc", [128, cw, INTER])
    nc.vector.tensor_scalar(out=gc, in0=PLAN[:, :, 160:168], scalar1=-30.0, scalar2=30.0,
                            op0=ALU.max, op1=ALU.min)
    eg = T("eg", [128, cw, INTER])
    nc.scalar.activation(eg, gc, ACTF.Exp, bias=0.0, scale=-1.0)
    dg = T("dg", [128, cw, INTER])
    nc.vector.tensor_scalar(out=dg, in0=eg, scalar1=1.0, scalar2=None, op0=ALU.add)
    nc.vector.reciprocal(out=G, in_=dg)

    # LMD init for nodes 0..7: LM = ln(|init|+1e-12), DIFF = VSIGN*|init| - LM
    vabs = T("vabs", [128, cw, INIT_SLOTS])
    nc.vector.tensor_scalar(out=vabs.bitcast(U32), in0=PLAN[:, :, 0:8].bitcast(U32),
                            scalar1=0x7FFFFFFF, scalar2=None, op0=ALU.bitwise_and)
    LM8 = LMD[:, 0, :, 0:8]
    nc.scalar.activation(LM8, vabs, ACTF.Ln, bias=eps)
    st["EPS"] = eps
    sg0 = T("sg0", [128, cw, INIT_SLOTS])
    nc.vector.tensor_tensor(out=sg0, in0=VSIGN[:, :, 0:8], in1=vabs, op=ALU.mult)
    nc.vector.tensor_tensor(out=LMD[:, 1, :, 0:8], in0=sg0, in1=LM8, op=ALU.subtract)

    # pv = prod VSIGN[0:8]
    pva = T("pva", [128, cw, 4])
    nc.vector.tensor_tensor(out=pva, in0=VSIGN[:, :, 0:4], in1=VSIGN[:, :, 4:8], op=ALU.mult)
    pvb = T("pvb", [128, cw, 2])
    nc.vector.tensor_tensor(out=pvb, in0=pva[:, :, 0:2], in1=pva[:, :, 2:4], op=ALU.mult)
    pv = T("pv", [128, cw])
    nc.vector.tensor_tensor(out=pv, in0=pvb[:, :, 0], in1=pvb[:, :, 1], op=ALU.mult)
    st["pv"] = pv

    # PVT[:, :, s] = prod_{j >= 8+s} VSIGN_init[j]
    nc.vector.tensor_copy(out=PVT[:, :, INTER - 1], in_=VSIGN[:, :, NN - 1])
    for j in range(INTER - 2, -1, -1):
        nc.vector.tensor_tensor(out=PVT[:, :, j], in0=PVT[:, :, j + 1],
                                in1=VSIGN[:, :, 8 + j], op=ALU.mult)
    return st


def _dag_step(nc, ns, gp, PLAN, st, c, s):
    cw = CW
    G, LMD, TL, PVT, AONE = st["G"], st["LMD"], st["TL"], st["PVT"], st["AONE"]
    v = INIT_SLOTS + s
    idx = v                     # node created this step
    O_s = PLAN[:, :, 32 + 16 * s: 32 + 16 * s + v]

    def T(nm, shape, dt=F32):
        return ns.tile(shape, dt, tag=f"{nm}{c}", name=f"{nm}{c}")

    def GT(nm, shape, dt=F32):
        return gp.tile(shape, dt, tag=f"{nm}{c}", name=f"{nm}{c}")

    # --- GpSimd: TP_s = prod_j (|O_sj|+1 masked) * PVT[s] ---
    oabs = GT("oabs", [128, cw, NN])
    nc.scalar.activation(oabs[:, :, 0:v], O_s, ACTF.Abs)
    obc = _ap(st["CONE"], 0, [[0, cw], [0, v]])
    nc.gpsimd.tensor_tensor(out=AONE[:, :, 0:v], in0=oabs[:, :, 0:v],
                            in1=obc, op=ALU.add)
    t8 = GT("t8", [128, cw, 8])
    nc.gpsimd.tensor_tensor(out=t8, in0=AONE[:, :, 0:8], in1=AONE[:, :, 8:16], op=ALU.mult)
    t4 = GT("t4", [128, cw, 4])
    nc.gpsimd.tensor_tensor(out=t4, in0=t8[:, :, 0:4], in1=t8[:, :, 4:8], op=ALU.mult)
    t2 = GT("t2", [128, cw, 2])
    nc.gpsimd.tensor_tensor(out=t2, in0=t4[:, :, 0:2], in1=t4[:, :, 2:4], op=ALU.mult)
    t1 = GT("t1", [128, cw])
    nc.gpsimd.tensor_tensor(out=t1, in0=t2[:, :, 0], in1=t2[:, :, 1], op=ALU.mult)
    TPs = GT("tp", [128, cw])
    nc.gpsimd.tensor_tensor(out=TPs, in0=t1, in1=PVT[:, :, s], op=ALU.mult)

    # --- DVE: R and SP ---
    tmp32 = T("tmp32", [128, 2, cw])        # planes [SP, R]
    SP = tmp32[:, 0, :]
    R = tmp32[:, 1, :]
    if s == 0:
        # TL row 0 = dot over init nodes for step 0 directly
        ml0 = T("ml0", [128, cw, 2, 8])
        o0 = _ap(PLAN, 32, [[168, cw], [0, 2], [1, 8]])
        l0 = _ap(LMD, 0, [[NN, cw], [NN * cw, 2], [1, 8]])
        nc.vector.tensor_tensor(out=ml0, in0=o0, in1=l0, op=ALU.mult)
        ab0 = T("ab0", [128, 2, cw])
        nc.vector.tensor_reduce(out=_ap(ab0, 0, [[1, cw], [cw, 2]]), in_=ml0,
                                op=ALU.add, axis=AXX)
        tls = ab0
    else:
        tls = TL[:, :, s, :]
    gq = T("gq", [128, cw])
    nc.vector.tensor_tensor(out=gq, in0=G[:, :, s], in1=tls[:, 1, :], op=ALU.mult)
    nc.vector.tensor_tensor(out=R, in0=tls[:, 0, :], in1=gq, op=ALU.add)
    nc.vector.tensor_tensor(out=SP, in0=st["pv"], in1=TPs, op=ALU.mult)

    # --- tanh pair: t = (1-e)/(1+e), e = exp(-2e4 * clip(x)) ---
    ec = T("ec", [128, 2, cw])
    nc.vector.tensor_scalar(out=ec, in0=tmp32, scalar1=-CLIP, scalar2=CLIP,
                            op0=ALU.max, op1=ALU.min)
    eo = T("eo", [128, 2, cw])
    nc.scalar.activation(eo, ec, ACTF.Exp, bias=0.0, scale=-2.0e4)
    Q = T("Q", [128, 4, cw])                # [lgs, er, lin, ar]
    minr = T("minr", [128, cw])
    nc.vector.tensor_scalar(out=minr, in0=R, scalar1=LOG_CLAMP, scalar2=None, op0=ALU.min)
    nc.scalar.activation(Q[:, 1, :], minr, ACTF.Exp)
    num = T("num", [128, 2, cw])
    nc.vector.tensor_scalar(out=num, in0=eo, scalar1=-1.0, scalar2=1.0,
                            op0=ALU.mult, op1=ALU.add)
    den = T("den", [128, 2, cw])
    nc.vector.tensor_scalar(out=den, in0=eo, scalar1=1.0, scalar2=None, op0=ALU.add)
    rd = T("rd", [128, 2, cw])
    nc.vector.reciprocal(out=rd, in_=den)
    nc.vector.tensor_tensor(out=_ap(Q, 0, [[2 * cw, 2], [1, cw]]), in0=num, in1=rd,
                            op=ALU.mult)
    nc.vector.tensor_scalar(out=Q[:, 3, :].bitcast(U32), in0=R.bitcast(U32),
                            scalar1=0x7FFFFFFF, scalar2=None, op0=ALU.bitwise_and)

    # --- Vs/Vm = x + G*(y-x) for pairs ([lgs,er],[lin,ar]) ---
    dd = T("dd", [128, 2, cw])
    nc.vector.tensor_tensor(out=dd, in0=Q[:, 2:4, :], in1=Q[:, 0:2, :], op=ALU.subtract)
    gd = T("gd", [128, 2, cw])
    gbc = _ap(G, s, [[0, 2], [INTER, cw]])
    nc.vector.tensor_tensor(out=gd, in0=gbc, in1=dd, op=ALU.mult)
    vsm = T("vsm", [128, 2, cw])
    nc.vector.tensor_tensor(out=vsm, in0=Q[:, 0:2, :], in1=gd, op=ALU.add)
    vs = vsm[:, 0, :]
    vm = vsm[:, 1, :]

    if s == INTER - 1:
        nc.vector.tensor_tensor(out=st["OUT"], in0=vs, in1=vm, op=ALU.mult)
        return

    sgnew = T("sgnew", [128, cw])
    nc.vector.tensor_tensor(out=sgnew, in0=vs, in1=vm, op=ALU.mult)
    LMn = LMD[:, 0, :, idx]
    nc.scalar.activation(LMn, vm, ACTF.Ln, bias=st["EPS"])
    nc.vector.tensor_tensor(out=LMD[:, 1, :, idx], in0=sgnew, in1=LMn, op=ALU.subtract)
    pvn = ns.tile([128, cw], F32, tag=f"pv{c}")
    nc.vector.tensor_tensor(out=pvn, in0=st["pv"], in1=vs, op=ALU.mult)
    st["pv"] = pvn

    if s == 0:
        # TL rows 1..7 = dot over init nodes (deferred so step 0 starts sooner);
        # split per LMD plane to stay within 3 free AP dims
        oall = _ap(PLAN, 32 + 16, [[NF, cw], [16, 7], [1, 8]])
        for pl in range(2):
            mall = T(f"mall{pl}", [128, cw, 7, 8])
            lpl = _ap(LMD, pl * NN * cw, [[NN, cw], [0, 7], [1, 8]])
            nc.vector.tensor_tensor(out=mall, in0=oall, in1=lpl, op=ALU.mult)
            nc.vector.tensor_reduce(
                out=_ap(TL, pl * INTER * cw + cw, [[1, cw], [cw, 7]]),
                in_=mall, op=ALU.add, axis=AXX)

    # on-create update: TL[:, :, s+1:, :] += O[:, s', idx] * LMDnew
    nfut = INTER - 1 - s
    tlm = T("tlm", [128, 2, 7, cw])
    ofut = _ap(PLAN, 32 + 16 * (s + 1) + idx, [[0, 2], [16, nfut], [NF, cw]])
    lnew = _ap(LMD, idx, [[NN * cw, 2], [0, nfut], [NN, cw]])
    nc.vector.tensor_tensor(out=tlm[:, :, 0:nfut, :], in0=ofut, in1=lnew, op=ALU.mult)
    tlslice = _ap(TL, (s + 1) * cw, [[INTER * cw, 2], [cw, nfut], [1, cw]])
    nc.vector.tensor_tensor(out=tlslice, in0=tlslice, in1=tlm[:, :, 0:nfut, :], op=ALU.add)


def _get_nc():
    if "nc" not in _CACHE:
        _CACHE["nc"] = _build()
    return _CACHE["nc"]


def _prep_inputs(hidden, W_init, b_init, W_op, b_op, W_gate, b_gate):
    hidden = np.ascontiguousarray(np.asarray(hidden, np.float32)).reshape(NTOK, H)
    Wcat = np.concatenate([np.asarray(W_init, np.float32),
                           np.asarray(W_op, np.float32),
                           np.asarray(W_gate, np.float32)], axis=0)   # [168, H]
    bcat = np.concatenate([np.asarray(b_init, np.float32),
                           np.asarray(b_op, np.float32),
                           np.asarray(b_gate, np.float32)])           # [168]

    WT = np.ascontiguousarray(Wcat.T)                                  # [H, 168]
    Wh = WT.astype(np.float16)
    Wl = ((WT - Wh.astype(np.float32)) * SCL).astype(np.float16)
    wt = np.concatenate([Wh, Wl], axis=1)                              # [H, 336]
    # partition-contiguous: [128, KCH*336]
    wt2 = np.ascontiguousarray(
        wt.reshape(KCH, 128, 336).transpose(1, 0, 2).reshape(128, KCH * 336))

    bh = bcat.astype(np.float16)
    bl = ((bcat - bh.astype(np.float32)) * SCL).astype(np.float16)
    bias = np.concatenate([bh, bl])[None, :]                           # [1, 336]

    in_maps = []
    for c in range(NCORES):
        shard = hidden[c * TPC:(c + 1) * TPC]                          # [2048, H]
        hT = np.ascontiguousarray(shard.T)                             # [H, 2048]
        fh = hT.astype(np.float16)
        fl = ((hT - fh.astype(np.float32)) * SCL).astype(np.float16)
        # [NTILE, 128, KCH*256]: partition p = H row within k-chunk,
        # free = (k, ct) with ct 0:128 = hi tokens, 128:256 = lo tokens
        comb = np.empty((NTILE, H, 256), np.float16)
        for i in range(NTILE):
            comb[i, :, 0:128] = fh[:, i * 128:(i + 1) * 128]
            comb[i, :, 128:256] = fl[:, i * 128:(i + 1) * 128]
        comb2 = np.ascontiguousarray(
            comb.reshape(NTILE, KCH, 128, 256).transpose(0, 2, 1, 3)
            .reshape(NTILE, 128, KCH * 256))
        in_maps.append({"hf": comb2, "wt": wt2, "bias": bias})
    return in_maps


def _run(in_maps, **kwargs):
    nc = _get_nc()
    return run_bass_kernel_spmd(nc, in_maps, core_ids=list(range(NCORES)), **kwargs)


def _assemble(results):
    out = np.empty((NTOK,), np.float32)
    for c in range(NCORES):
        out[c * TPC:(c + 1) * TPC] = results[c]["out"].T.reshape(TPC)
    return out.reshape(B, T)


def kernel(**inputs):
    in_maps = _prep_inputs(**inputs)
    res = _run(in_maps)
    return _assemble(res.results)


def kernel_traced(**inputs):
    """Like kernel() but with NTFF tracing; returns (output, BassKernelResults)."""
    in_maps = _prep_inputs(**inputs)
    res = _run(in_maps, trace=True)
    return _assemble(res.results), res
